# revision 1
# baseline (speedup 1.0000x reference)
"""CisAttentionLayer Trainium2 kernel — 8-core SPMD via bass/Tile.

Sharding: core = (batch b, gene-half gh). Each of the 8 cores computes the
full attention layer for 512 genes of one batch: Q/K/V projections, masked
softmax over 4096 SNPs, output projection and layernorm. No collectives.

Layout strategy (all matmuls contract over the SBUF partition dim):
  - Host feeds transposed operands: kvT [D_SNP, S], qT [D_SNP, GL], w*.T.
  - K^T [D,S] and Q^T [D,GL] are produced on-device in fp16; scores are
    computed transposed (scoresT [s, g]) so attn@V needs no transposes.
  - softmax: no max-subtraction (scores are small once row-constant terms
    are dropped); pad mask folded into the exp per-partition bias; cis mask
    applied as a {0,1} fp16 post-multiply; row sums come free from an
    all-ones 65th column appended to V (ones row of each head accumulator).
  - The per-gene 1/rowsum and the per-head output projection are combined
    in phase 3: final[g,:] = sum_h (rawT_h.T @ woT_h) * z_h[g] + wo_b, then
    layernorm.

Softmax math note: scores differ from the reference by row-constant terms
(q·dk, dq·dk are independent of the SNP index), which cancel in softmax, so
K is projected WITHOUT the dk shift — keeps fp16 ranges tiny and exact.
"""
import numpy as np
import concourse.bass as bass
import concourse.tile as tile
from concourse import mybir
from concourse.bass_utils import run_bass_kernel_spmd
from concourse.vector_clock import ScopedClock

B, G, S, D, H, DK = 4, 1024, 4096, 512, 8, 64
GL = G // 2            # genes per core
N_CORES = 8
SCALE = 1.0 / np.sqrt(DK).astype(np.float32)  # 0.125 (TEMPERATURE=1)
NEG_PAD = -30000.0     # pad-mask bias inside exp (exp underflows to 0)

F32 = mybir.dt.float32
F32R = mybir.dt.float32r
F16 = mybir.dt.float16
AF = mybir.ActivationFunctionType
ALU = mybir.AluOpType


# ---------------------------------------------------------------------------
# Tile compat: this container's walrus rejects >1 sync wait per instruction.
# ---------------------------------------------------------------------------
def _split_sync_waits(nc):
    for f in nc.m.functions:
        for bb in f.blocks:
            idx = 0
            while idx < len(bb.instructions):
                inst = bb.instructions[idx]
                si = inst.sync_info
                if si is not None and len(si.on_wait) > 1:
                    waits = list(si.on_wait)
                    for w in waits[:-1]:
                        nop = mybir.InstNoOp(
                            name=nc.get_next_instruction_name(),
                            sync_info=mybir.SyncInfo(on_wait=[w], on_update=[]),
                            bass_nofuse=True,
                            engine=inst.engine,
                        )
                        nc.register_instruction(nop)
                        bb.instructions.insert(idx, nop)
                        idx += 1
                    inst.sync_info = mybir.SyncInfo(
                        on_wait=[waits[-1]], on_update=list(si.on_update)
                    )
                idx += 1


class _SafeTileContext(tile.TileContext):
    def _drain_and_barrier(self, tick_clock, wait_clock):
        drain_inst = self.nc.sync.drain()
        wait_clock.add_sem_waits(
            drain_inst.ins, ScopedClock({None: tick_clock.global_clock})
        )
        si = drain_inst.ins.sync_info
        if si is not None and len(si.on_wait) > 1:
            waits = list(si.on_wait)
            drain_inst.ins.sync_info = mybir.SyncInfo(
                on_wait=[waits[0]], on_update=list(si.on_update)
            )
            for w in waits[1:]:
                extra = self.nc.sync.drain()
                extra.ins.sync_info = mybir.SyncInfo(on_wait=[w], on_update=[])
        self.nc.all_engine_barrier()
        assert self.sems is not None
        popped = self.nc._tile_sem_poison_stack.pop()
        assert popped is self._sem_poison
        self.nc.clear_and_free_semaphores(list(self.sems.allocated().values()))
        self.nc.all_engine_barrier()


# ---------------------------------------------------------------------------
# Kernel build
# ---------------------------------------------------------------------------
def _bcast_ap(dram_t, parts, free):
    """Partition-broadcast DMA source AP for a [1, free] dram tensor."""
    return bass.AP(tensor=dram_t.ap().tensor, offset=0, ap=[[0, parts], [1, free]])


def build_nc():
    nc = bass.Bass()
    kvT_d = nc.dram_tensor("kvT", [D, S], F16, kind="ExternalInput")
    qT_d = nc.dram_tensor("qT", [D, GL], F16, kind="ExternalInput")
    wqT_d = nc.dram_tensor("wqT", [D, D], F16, kind="ExternalInput")
    wkT_d = nc.dram_tensor("wkT", [D, D], F16, kind="ExternalInput")
    wvT_d = nc.dram_tensor("wvT", [D, D], F16, kind="ExternalInput")
    woT_d = nc.dram_tensor("woT", [D, D], F32R, kind="ExternalInput")
    qb_d = nc.dram_tensor("qbias", [D, 1], F32, kind="ExternalInput")
    ob_d = nc.dram_tensor("obias", [1, D], F32, kind="ExternalInput")
    lng_d = nc.dram_tensor("lng", [1, D], F32, kind="ExternalInput")
    lnb_d = nc.dram_tensor("lnb", [1, D], F32, kind="ExternalInput")
    pb_d = nc.dram_tensor("padb", [S, 1], F32, kind="ExternalInput")
    eye_d = nc.dram_tensor("eye8", [H, H], F32, kind="ExternalInput")
    cis_d = nc.dram_tensor("cisT", [S, GL], F16, kind="ExternalInput")
    out_d = nc.dram_tensor("out", [GL, D], F32, kind="ExternalOutput")

    with _SafeTileContext(nc) as tc:
        with tc.tile_pool(name="const", bufs=1) as const, \
             tc.tile_pool(name="res", bufs=1) as res, \
             tc.tile_pool(name="psum", bufs=1, space="PSUM") as psum:
            # ---- constants (small, SWDGE) ----
            qb = const.tile([128, 4], F32, tag="qb")
            nc.gpsimd.dma_start(out=qb, in_=bass.AP(
                tensor=qb_d.ap().tensor, offset=0, ap=[[1, 128], [128, 4]]))
            pb = const.tile([128, 32], F32, tag="pb")
            nc.gpsimd.dma_start(out=pb, in_=bass.AP(
                tensor=pb_d.ap().tensor, offset=0, ap=[[1, 128], [128, 32]]))
            epsT = const.tile([128, 1], F32, tag="eps")
            nc.vector.memset(epsT, 1e-5)

            # ---- resident tensors ----
            KT = [res.tile([128, S], F16, tag=f"kt{i}", name=f"KT{i}")
                  for i in range(4)]
            QT = [res.tile([128, GL], F16, tag=f"qt{i}", name=f"QT{i}")
                  for i in range(4)]
            VA = res.tile([128, 32, 520], F16, tag="va")
            CIS = res.tile([128, 32, GL], F16, tag="cis")
            OT = res.tile([65, H, GL], F32R, tag="ot")
            rs_sb = res.tile([128, 4, H], F32, tag="rs")
            zrec = res.tile([128, 4, H], F32, tag="z")
            rs_all = res.tile([H, GL], F32, tag="rsall")
            nc.vector.memset(rs_all, 0.0)

            # cis mask: SWDGE in 8 slabs, overlaps phase 1 compute
            for c8 in range(8):
                nc.gpsimd.dma_start(
                    out=CIS[:, c8 * 4:(c8 + 1) * 4, :],
                    in_=bass.AP(tensor=cis_d.ap().tensor,
                                offset=c8 * 4 * 128 * GL,
                                ap=[[GL, 128], [128 * GL, 4], [1, GL]]))
            # late-phase constants (queued behind the cis mask on SWDGE)
            lngB = const.tile([128, D], F32, tag="lng")
            nc.gpsimd.dma_start(out=lngB, in_=_bcast_ap(lng_d, 128, D))
            lnbB = const.tile([128, D], F32, tag="lnb")
            nc.gpsimd.dma_start(out=lnbB, in_=_bcast_ap(lnb_d, 128, D))
            eye8 = const.tile([H, H], F32, tag="eye8")
            nc.gpsimd.dma_start(out=eye8, in_=eye_d.ap())

            # ---- attention helpers (used inline in phase 1 for pair 0) ----
            def attn_chunk(pair, accs, sc, p2):
                blk = pair
                pss = psum.tile([128, 1024], F32, tag="pss", bufs=2, name="pss")
                for j in range(2):
                    off = j * 64
                    nc.tensor.matmul(
                        pss[:, j * 512:(j + 1) * 512],
                        KT[blk][off:off + 64, sc * 128:(sc + 1) * 128],
                        QT[blk][off:off + 64, :],
                        start=True, stop=True)
                et = p2.tile([128, 1024], F16, tag="et", bufs=3, name="et")
                nc.scalar.activation(et, pss, AF.Exp,
                                     bias=pb[:, sc:sc + 1], scale=SCALE)
                at = p2.tile([128, 1024], F16, tag="at", bufs=3, name="at")
                cis_sc = CIS[:, sc, :]
                cis_b = bass.AP(tensor=cis_sc.tensor, offset=cis_sc.offset,
                                ap=[cis_sc.ap[0], [0, 2], cis_sc.ap[1]])
                nc.vector.tensor_tensor(
                    out=at.rearrange("p (j g) -> p j g", g=512),
                    in0=et.rearrange("p (j g) -> p j g", g=512),
                    in1=cis_b, op=ALU.mult)
                for j in range(2):
                    h = pair * 2 + j
                    nc.tensor.matmul(
                        accs[j], VA[:, sc, h * 65:(h + 1) * 65],
                        at[:, j * 512:(j + 1) * 512],
                        start=(sc == 0), stop=(sc == 31))

            def flush_pair(pair, accs, FACC, wohs, ln_fn=None):
                for j in range(2):
                    h = pair * 2 + j
                    nc.vector.tensor_copy(OT[:, h, :], accs[j])
                    nc.gpsimd.dma_start(out=rs_all[h:h + 1, :],
                                        in_=OT[64:65, h, :])
                for t in range(4):
                    pstt = psum.tile([128, D], F32, tag="psp", bufs=2, name="pst")
                    pst = pstt[:, 0:H]
                    nc.tensor.matmul(pst, rs_all[:, t * 128:(t + 1) * 128],
                                     eye8, start=True, stop=True)
                    h0 = pair * 2
                    nc.vector.reciprocal(zrec[:, t, h0:h0 + 2],
                                         pst[:, h0:h0 + 2])
                    for j in range(2):
                        h = pair * 2 + j
                        psp = psum.tile([128, D], F32, tag="psp", bufs=2,
                                        name="psp")
                        nc.tensor.matmul(
                            psp, OT[0:64, h, t * 128:(t + 1) * 128],
                            wohs[h], start=True, stop=True)
                        nc.vector.scalar_tensor_tensor(
                            out=FACC[t], in0=psp,
                            scalar=zrec[:, t, h:h + 1],
                            in1=FACC[t], op0=ALU.mult, op1=ALU.add)
                    if ln_fn is not None:
                        ln_fn(t)

            # =========== phase 1: projections ===========
            p2_cm = tc.tile_pool(name="p2", bufs=2)
            p2 = p2_cm.__enter__()
            with tc.tile_pool(name="p1", bufs=1) as p1:
                wk = [p1.tile([128, D], F16, tag=f"wk{i}", name=f"wkb{i}")
                      for i in range(4)]
                wv = [p1.tile([128, D], F16, tag=f"wv{i}", name=f"wvb{i}")
                      for i in range(4)]
                for i in range(4):
                    nc.sync.dma_start(out=wk[i], in_=wkT_d[i * 128:(i + 1) * 128, :])
                kv0 = [p1.tile([128, 512], F16, tag=f"kvz{d}",
                               name=f"kvz{d}") for d in range(4)]
                for d in range(4):
                    nc.sync.dma_start(out=kv0[d], in_=kvT_d[d * 128:(d + 1) * 128, 0:512])
                for i in range(4):
                    nc.sync.dma_start(out=wv[i], in_=wvT_d[i * 128:(i + 1) * 128, :])

                def do_eighth(e, kvq):
                    # K^T tiles (fp16, per-partition bias; dk dropped — see note)
                    for dout in range(4):
                        ps = psum.tile([128, 512], F32, tag="psp", bufs=2, name="psk")
                        for dblk in range(4):
                            nc.tensor.matmul(
                                ps, wk[dblk][:, dout * 128:(dout + 1) * 128],
                                kvq[dblk], start=(dblk == 0), stop=(dblk == 3))
                        c0 = e * 512
                        nc.scalar.activation(
                            KT[dout][:, c0:c0 + 512], ps, AF.Identity,
                            bias=0.0, scale=1.0)
                    # V tiles -> VA strided (+ wv_b via K=1 matmul, ones col)
                    for sc in range(4):
                        ps = psum.tile([128, 512], F32, tag="psp", bufs=2, name="psv")
                        for dblk in range(4):
                            nc.tensor.matmul(
                                ps, kvq[dblk][:, sc * 128:(sc + 1) * 128],
                                wv[dblk], start=(dblk == 0), stop=(dblk == 3))
                        gsc = e * 4 + sc
                        dstv = VA[:, gsc, :].rearrange("p (h c) -> p h c", c=65)
                        nc.vector.tensor_copy(
                            dstv[:, :, 0:64],
                            ps.rearrange("p (h c) -> p h c", c=64))
                        nc.vector.memset(dstv[:, :, 64:65], 1.0)

                # Q^T blocks (kv eighth 0 already queued on HWDGE first)
                with tc.tile_pool(name="p1q", bufs=1) as p1q:
                    wq = [p1q.tile([128, D], F16, tag=f"wq{i}", name=f"wqb{i}")
                          for i in range(4)]
                    qTs = [p1q.tile([128, GL], F16, tag=f"q{i}", name=f"qTs{i}")
                           for i in range(4)]
                    for i in range(4):
                        sl = slice(i * 128, (i + 1) * 128)
                        nc.sync.dma_start(out=wq[i], in_=wqT_d[sl, :])
                        nc.sync.dma_start(out=qTs[i], in_=qT_d[sl, :])
                    do_eighth(0, kv0)
                    for dout in range(4):
                        ps = psum.tile([128, GL], F32, tag="psp", bufs=2, name="psq")
                        for dblk in range(4):
                            nc.tensor.matmul(
                                ps, wq[dblk][:, dout * 128:(dout + 1) * 128],
                                qTs[dblk], start=(dblk == 0), stop=(dblk == 3))
                        nc.scalar.activation(QT[dout], ps, AF.Identity,
                                             bias=qb[:, dout:dout + 1], scale=1.0)

                accs0 = [psum.tile([65, 512], F32, tag=f"acc{j}",
                                   name=f"acc0{j}") for j in range(2)]
                p1kv_cm = tc.tile_pool(name="p1kv", bufs=2)
                p1kv = p1kv_cm.__enter__()

                def load_eighth(e):
                    kvq = [p1kv.tile([128, 512], F16, tag=f"kv{d}",
                                     name=f"kvq{d}") for d in range(4)]
                    for d in range(4):
                        nc.sync.dma_start(
                            out=kvq[d],
                            in_=kvT_d[d * 128:(d + 1) * 128,
                                      e * 512:(e + 1) * 512])
                    return kvq

                for sc in range(4):
                    attn_chunk(0, accs0, sc, p2)
                for e in range(1, 8):
                    do_eighth(e, load_eighth(e))
                    for sc in range(e * 4, e * 4 + 4):
                        attn_chunk(0, accs0, sc, p2)
                p1kv_cm.__exit__(None, None, None)

            # =========== phase 2+3: attention, fused out-proj ===========
            with tc.tile_pool(name="res2", bufs=1) as res2, \
                 tc.tile_pool(name="p3", bufs=2) as p3:
                FACC = [res2.tile([128, D], F32, tag=f"fa{i}", name=f"FACC{i}")
                        for i in range(4)]
                wohs = [res2.tile([64, D], F32R, tag=f"wo{i}", name=f"woh{i}")
                        for i in range(H)]
                for i in range(H):
                    nc.sync.dma_start(out=wohs[i],
                                      in_=woT_d[i * 64:(i + 1) * 64, :])
                for t in range(4):
                    nc.gpsimd.dma_start(out=FACC[t], in_=_bcast_ap(ob_d, 128, D))

                def ln_tile(t):
                    stats = p3.tile([128, 6], F32, tag="st", name="st")
                    nc.vector.bn_stats(out=stats, in_=FACC[t])
                    mv = p3.tile([128, 2], F32, tag="mv", name="mv")
                    nc.vector.bn_aggr(out=mv, in_=stats)
                    std = p3.tile([128, 1], F32, tag="sd", name="sd")
                    nc.scalar.activation(std, mv[:, 1:2], AF.Sqrt,
                                         bias=epsT, scale=1.0)
                    rstd = p3.tile([128, 1], F32, tag="rsd", name="rsd")
                    nc.vector.reciprocal(rstd, std)
                    t1 = p3.tile([128, D], F32, tag="t1", name="t1")
                    nc.vector.tensor_scalar(out=t1, in0=FACC[t],
                                            scalar1=mv[:, 0:1], scalar2=rstd,
                                            op0=ALU.subtract, op1=ALU.mult)
                    t2 = p3.tile([128, D], F32, tag="t2", name="t2")
                    nc.gpsimd.tensor_tensor(out=t2, in0=t1, in1=lngB, op=ALU.mult)
                    t3 = p3.tile([128, D], F32, tag="t3", name="t3")
                    nc.gpsimd.tensor_tensor(out=t3, in0=t2, in1=lnbB, op=ALU.add)
                    nc.sync.dma_start(out=out_d[t * 128:(t + 1) * 128, :], in_=t3)

                flush_pair(0, accs0, FACC, wohs)
                for pair in range(1, 4):
                    accs = [psum.tile([65, 512], F32, tag=f"acc{j}",
                                      name=f"accs{j}") for j in range(2)]
                    for sc in range(32):
                        attn_chunk(pair, accs, sc, p2)
                    flush_pair(pair, accs, FACC, wohs,
                               ln_fn=ln_tile if pair == 3 else None)
            p2_cm.__exit__(None, None, None)

    _split_sync_waits(nc)
    nc.finalize()
    return nc

# ---------------------------------------------------------------------------
# Host-side sharding / unsharding
# ---------------------------------------------------------------------------
def make_in_maps(queries, keys_values, dq, dk, mask, cis_mask,
                 wq_w, wq_b, wk_w, wk_b, wv_w, wv_b, wo_w, wo_b, ln_g, ln_b):
    f32 = np.float32
    wqT = np.ascontiguousarray(wq_w.T).astype(np.float16)
    wkT = np.ascontiguousarray(wk_w.T).astype(np.float16)
    wvT = np.ascontiguousarray(wv_w.T).astype(np.float16)
    woT = np.ascontiguousarray(wo_w.astype(f32).T)
    eye8 = np.eye(H, dtype=f32)
    lng = ln_g.astype(f32).reshape(1, D)
    lnb = ln_b.astype(f32).reshape(1, D)
    # wv_b's effect on the normalized attention output is a constant per
    # head (attn rows sum to 1), so it folds into the output bias exactly.
    ob = (wo_b.astype(f32) + wv_b.astype(f32) @ wo_w.astype(f32).T).reshape(1, D)
    cisT = np.ascontiguousarray(cis_mask.T).astype(np.float16)  # [S, G]

    in_maps = []
    for core in range(N_CORES):
        b, gh = core // 2, core % 2
        gsl = slice(gh * GL, (gh + 1) * GL)
        in_maps.append(dict(
            kvT=np.ascontiguousarray(keys_values[b].T).astype(np.float16),
            qT=np.ascontiguousarray(queries[b, gsl, :].T).astype(np.float16),
            wqT=wqT, wkT=wkT, wvT=wvT, woT=woT,
            qbias=(wq_b.astype(f32) + dq[b, 0].astype(f32)).reshape(D, 1),
            obias=ob, lng=lng, lnb=lnb,
            padb=np.where(mask[b] == 0, np.float32(NEG_PAD),
                          np.float32(0.0)).reshape(S, 1),
            eye8=eye8,
            cisT=np.ascontiguousarray(cisT[:, gsl]),
        ))
    return in_maps


_CACHE = {}


def _run_in_maps(in_maps):
    if "nc" not in _CACHE:
        _CACHE["nc"] = build_nc()
    res = run_bass_kernel_spmd(_CACHE["nc"], in_maps,
                               core_ids=list(range(N_CORES)))
    return [r["out"] for r in res.results]


def _child_run(in_maps, q):
    try:
        q.put(("ok", _run_in_maps(in_maps)))
    except Exception as e:  # noqa: BLE001
        q.put(("err", repr(e)))


def kernel(**inputs):
    in_maps = make_in_maps(**inputs)
    outs = None
    try:
        outs = _run_in_maps(in_maps)
    except Exception:
        # A failed NEFF exec leaves this process's device client unrecoverable;
        # a fresh process (with the NEFF already cached) succeeds. Retry there.
        import multiprocessing as mp
        ctx = mp.get_context("spawn")
        last = None
        for _ in range(3):
            q = ctx.Queue()
            proc = ctx.Process(target=_child_run, args=(in_maps, q))
            proc.start()
            status, payload = q.get()
            proc.join()
            if status == "ok":
                outs = payload
                break
            last = payload
        if outs is None:
            raise RuntimeError(f"kernel failed after retries: {last}")
    out = np.empty((B, G, D), np.float32)
    for core in range(N_CORES):
        b, gh = core // 2, core % 2
        out[b, gh * GL:(gh + 1) * GL, :] = outs[core]
    return out

